# revision 41
# baseline (speedup 1.0000x reference)
"""Causal single-head attention (B=4, T=4096, C=512, D=64) on 8 TRN2 NeuronCores.

Sharding: core c -> (batch b = c // 2, parity P = c % 2).  Each batch's 32
q-tiles (128 rows each) are striped by parity: core (b, P) owns global q-tiles
k = 2j + P, j = 0..15.  Slot j's causal kv extent is padded to 256*(j+1) keys
(uniform across parities, +3% work) and the last 256 key columns get a
parity-specific additive mask fed as input data, so one SPMD program serves
all 8 cores.

Per-core dataflow:
  phase 1: K^T|V^T from a single W-stationary matmul per x^T chunk (Wk and Wv
           stacked into one 128-wide stationary operand; C=512 contracted in
           4 chunks of 128); V^T is PE-transposed into fp32 V[S, D+1] tiles
           whose last column is 1.0 (gives row sums for free during AV).
           Q^T/K^T are mirrored into the upper 64 SBUF partitions so the
           scores matmuls run 2x row-tiled (K=64 uses half the PE array; two
           concurrent 64-row tiles).  K-hat = [K^T; ones] and Q-hat =
           [Q^T; -m_row] buffers serve the transposed-scores matmul.
  phase 2 (flash, per slot j, groups of up to 1536 keys):
           1. scores S = Q_j K^T on PE (row-tiled, 512-wide PSUM chunks),
              additive -1e30 mask on the slot's last 256 columns, DVE row-max
              -> running max m (these scores are used ONLY for the max).
           2. -m written as a [1,128] row into Q-hat partition 64 (PE
              transpose of m via identity matmul + DVE negate-copy).
           3. S^T - m computed directly on PE via the 65-deep contraction
              [K^T; 1]^T [Q^T; -m] -> one ACT exp (scale=8) writes A^T
              straight to SBUF (no PSUM->SBUF copy pass, no PE transposes
              of A), masked via a transposed mask on the last two blocks.
           4. AV: po[128, 65] += A^T_block^T V-hat_block on PE; column 64
              accumulates the row sums l.  Running rescale of (O, l) by
              exp(8*(m_old - m_new)) on DVE; final y = O / l.
"""

import numpy as np

B, T, C, D = 4, 4096, 512, 64
P128 = 128
NSLOT = 16          # q-tile slots per core
TQ = NSLOT * P128   # 2048 q rows per core
NEG = -1.0e30
GRP = 1536

_CACHED = {}


def _build(use_dma_t=False, rowtile=True, st_mode=True, grp=GRP):
    import concourse.bass as bass
    import concourse.mybir as mybir
    from concourse import bacc
    from concourse.tile import TileContext
    from concourse.masks import make_identity

    f32 = mybir.dt.float32
    bf16 = mybir.dt.bfloat16
    AX = mybir.AxisListType.X
    ALU = mybir.AluOpType
    ACTF = mybir.ActivationFunctionType

    nc = bacc.Bacc("TRN2", target_bir_lowering=False, debug=False,
                   enable_asserts=False, num_devices=8)

    xT = nc.dram_tensor("xT", [C, T], f32, kind="ExternalInput").ap()
    xTq = nc.dram_tensor("xTq", [C, TQ], f32, kind="ExternalInput").ap()
    wq = nc.dram_tensor("wq", [C, D], f32, kind="ExternalInput").ap()
    wk = nc.dram_tensor("wk", [C, D], f32, kind="ExternalInput").ap()
    wv = nc.dram_tensor("wv", [C, D], f32, kind="ExternalInput").ap()
    bq = nc.dram_tensor("bq", [D, 1], f32, kind="ExternalInput").ap()
    bk = nc.dram_tensor("bk", [D, 1], f32, kind="ExternalInput").ap()
    bv = nc.dram_tensor("bv", [D, 1], f32, kind="ExternalInput").ap()
    maskp = nc.dram_tensor("maskp", [P128, 512], f32, kind="ExternalInput").ap()
    maskpT = nc.dram_tensor("maskpT", [P128, 256], f32, kind="ExternalInput").ap()
    y = nc.dram_tensor("y", [TQ, D], f32, kind="ExternalOutput").ap()
    DV = D + 1 if st_mode else D  # V tiles carry a ones column in st_mode

    with TileContext(nc) as tc:
        with (
            tc.tile_pool(name="singles", bufs=1) as singles,
            tc.tile_pool(name="xin", bufs=3) as xin,
            tc.tile_pool(name="work", bufs=2) as work,
            tc.tile_pool(name="small", bufs=3) as small,
            tc.tile_pool(name="ps_s", bufs=2, space="PSUM") as ps_s,
            tc.tile_pool(name="ps_s2", bufs=1, space="PSUM") as ps_s2,
            tc.tile_pool(name="ps_t", bufs=1, space="PSUM") as ps_t,
            tc.tile_pool(name="ps_o", bufs=1, space="PSUM") as ps_o,
            tc.tile_pool(name="ps_p", bufs=1, space="PSUM") as ps_p,
        ):
            # ---- resident constants (SWDGE loads) ----
            wqs = singles.tile([P128, 4, D], f32, tag="wqs")
            wkv = singles.tile([P128, 4, 2 * D], f32, tag="wkv")
            nc.gpsimd.dma_start(out=wqs, in_=wq.rearrange("(c p) d -> p c d", p=P128))
            nc.gpsimd.dma_start(out=wkv[:, :, :D],
                                in_=wk.rearrange("(c p) d -> p c d", p=P128))
            nc.gpsimd.dma_start(out=wkv[:, :, D:],
                                in_=wv.rearrange("(c p) d -> p c d", p=P128))
            bqs = singles.tile([D, 1], f32, tag="bqs")
            bks = singles.tile([D, 1], f32, tag="bks")
            bvs = singles.tile([D, 1], f32, tag="bvs")
            nc.gpsimd.dma_start(out=bqs, in_=bq)
            nc.gpsimd.dma_start(out=bks, in_=bk)
            nc.gpsimd.dma_start(out=bvs, in_=bv)
            msk = singles.tile([P128, 512], f32, tag="msk")
            nc.gpsimd.dma_start(out=msk, in_=maskp)
            if st_mode:
                mskT = singles.tile([P128, 256], f32, tag="mskT")
                nc.gpsimd.dma_start(out=mskT, in_=maskpT)
                identf = singles.tile([P128, P128], f32, tag="identf")
                make_identity(nc, identf)
            else:
                mskT = identf = None

            QP = P128 if rowtile else D
            QTo = singles.tile([QP, TQ], f32, tag="QTo")
            KT = singles.tile([QP, T], f32, tag="KT")
            Vsb = singles.tile([P128, (T // P128) * DV], f32, tag="Vsb")
            if st_mode:
                # K-hat: [K^T; ones] and Q-hat: [Q^T; -m_row] for the shifted
                # transposed-scores matmul (rank-1 max subtraction in-contraction)
                KH = singles.tile([D + 1, T], f32, tag="KH")
                QH = singles.tile([D + 1, TQ], f32, tag="QH")
                nc.vector.memset(KH[D:D + 1, :], 1.0)
                nc.vector.memset(Vsb, 1.0)   # ones column survives V writes

            # ---- phase 1: projections ----
            for t8 in range(T // 512):
                xt = xin.tile([P128, 4, 512], f32, tag="xt")
                nc.gpsimd.dma_start(
                    out=xt,
                    in_=xT[:, t8 * 512:(t8 + 1) * 512].rearrange(
                        "(c p) n -> p c n", p=P128),
                )
                kvps = ps_p.tile([2 * D, 512], f32, tag="pp")
                for c in range(4):
                    nc.tensor.matmul(kvps, wkv[:, c, :], xt[:, c, :],
                                     start=(c == 0), stop=(c == 3))
                nc.scalar.activation(KT[:D, t8 * 512:(t8 + 1) * 512], kvps[:D, :],
                                     ACTF.Identity, bias=bks, scale=1.0)
                vtmp = work.tile([D, 512], f32, tag="vtmp")
                nc.scalar.activation(vtmp, kvps[D:, :], ACTF.Identity,
                                     bias=bvs, scale=1.0)
                # transpose V^T [64, 128] blocks -> bf16 V [128, 64] tiles
                for i in range(4):
                    t = t8 * 4 + i
                    if use_dma_t:
                        nc.sync.dma_start(
                            out=Vsb[:, t * DV:t * DV + D],
                            in_=vtmp[:, i * P128:(i + 1) * P128], transpose=True)
                    else:
                        pt = ps_t.tile([P128, 512], f32, tag="pt")
                        nc.tensor.transpose(pt[:, :D],
                                            vtmp[:, i * P128:(i + 1) * P128],
                                            identf[:D, :D])
                        nc.vector.tensor_copy(Vsb[:, t * DV:t * DV + D], pt[:, :D])
            # Q^T from xTq (own 2048 rows)
            for t8 in range(TQ // 512):
                xt = xin.tile([P128, 4, 512], f32, tag="xt")
                nc.gpsimd.dma_start(
                    out=xt,
                    in_=xTq[:, t8 * 512:(t8 + 1) * 512].rearrange(
                        "(c p) n -> p c n", p=P128),
                )
                qps = ps_p.tile([D, 512], f32, tag="pp")
                for c in range(4):
                    nc.tensor.matmul(qps, wqs[:, c, :], xt[:, c, :],
                                     start=(c == 0), stop=(c == 3))
                nc.scalar.activation(QTo[:D, t8 * 512:(t8 + 1) * 512], qps,
                                     ACTF.Identity, bias=bqs, scale=1.0)
            if rowtile:
                # mirror Q^T/K^T into the upper 64 partitions for row tiling
                nc.gpsimd.dma_start(out=KT[D:2 * D, :], in_=KT[:D, :])
                nc.gpsimd.dma_start(out=QTo[D:2 * D, :], in_=QTo[:D, :])
            if st_mode:
                nc.gpsimd.dma_start(out=KH[:D, :], in_=KT[:D, :])
                nc.gpsimd.dma_start(out=QH[:D, :], in_=QTo[:D, :])

            # ---- phase 2: per-slot flash attention ----
            for j in range(NSLOT):
                ncols = 256 * (j + 1)
                groups = []
                off = 0
                while off < ncols:
                    groups.append((off, min(grp, ncols - off)))
                    off += grp

                mrun = small.tile([P128, 1], f32, tag="mrun")
                lrun = small.tile([P128, 1], f32, tag="lrun")
                Oacc = small.tile([P128, D], f32, tag="Oacc")

                for gi, (off, w) in enumerate(groups):
                    last = (gi == len(groups) - 1)
                    subs = list(range(0, w, 512))
                    mgp = small.tile([P128, 4], f32, tag="mgp")
                    for si, soff in enumerate(subs):
                        sw = min(512, w - soff)
                        half = ((off + soff) // 512) % 2 if rowtile else 0
                        pbase = half * D
                        ps = ps_s.tile([P128, 512], f32, tag="ps")
                        nc.tensor.matmul(
                            ps[:, :sw],
                            QTo[pbase:pbase + D, j * P128:(j + 1) * P128],
                            KT[pbase:pbase + D, off + soff:off + soff + sw],
                            start=True, stop=True)
                        if last and si == len(subs) - 1:
                            nc.vector.tensor_add(ps[:, sw - 256:sw],
                                                 ps[:, sw - 256:sw],
                                                 msk[:, 256:512])
                        nc.vector.reduce_max(mgp[:, si:si + 1], ps[:, :sw], axis=AX)
                    mg = small.tile([P128, 1], f32, tag="mg")
                    if len(subs) > 1:
                        nc.vector.reduce_max(mg, mgp[:, :len(subs)], axis=AX)
                    else:
                        nc.vector.tensor_copy(mg, mgp[:, :1])
                    if gi == 0:
                        nc.vector.tensor_copy(mrun, mg)
                    else:
                        mnew = small.tile([P128, 1], f32, tag="mnew")
                        nc.vector.tensor_max(mnew, mrun, mg)
                        mdiff = small.tile([P128, 1], f32, tag="mdiff")
                        nc.vector.tensor_sub(mdiff, mrun, mnew)
                        cstep = small.tile([P128, 1], f32, tag="cstep")
                        nc.scalar.activation(cstep, mdiff, ACTF.Exp,
                                             bias=0.0, scale=8.0)
                        nc.vector.tensor_copy(mrun, mnew)
                    nblk = w // P128
                    base = off // P128
                    AT = work.tile([P128, grp], f32, tag="AT")
                    po = ps_o.tile([P128, DV], f32, tag="po")
                    if st_mode:
                        # -m_new as a [1, 128] row at QH partition 64 (via PE)
                        pm = ps_t.tile([P128, P128], f32, tag="pt")
                        nc.tensor.matmul(pm[:1, :P128], mrun, identf,
                                         start=True, stop=True)
                        nc.vector.tensor_scalar_mul(
                            QH[D:D + 1, j * P128:(j + 1) * P128],
                            pm[:1, :P128], -1.0)
                        # shifted transposed scores: S^T - m  (65-contraction)
                        ps2 = ps_s2.tile([P128, grp], f32, tag="ps2")
                        for i in range(nblk):
                            nc.tensor.matmul(
                                ps2[:, i * P128:(i + 1) * P128],
                                KH[:, off + i * P128:off + (i + 1) * P128],
                                QH[:, j * P128:(j + 1) * P128],
                                start=True, stop=True)
                        if last:
                            nc.vector.tensor_add(
                                ps2[:, w - 256:w - P128], ps2[:, w - 256:w - P128],
                                mskT[:, 0:P128])
                            nc.vector.tensor_add(
                                ps2[:, w - P128:w], ps2[:, w - P128:w],
                                mskT[:, P128:256])
                        nc.scalar.activation(AT[:, :w], ps2[:, :w], ACTF.Exp,
                                             bias=0.0, scale=8.0)
                    else:
                        raise NotImplementedError("non-st_mode path removed")
                    for i in range(nblk):
                        nc.tensor.matmul(po, AT[:, i * P128:(i + 1) * P128],
                                         Vsb[:, (base + i) * DV:(base + i) * DV + DV],
                                         start=(i == 0), stop=(i == nblk - 1))

                    lg_ap = po[:, D:D + 1]
                    if gi == 0:
                        nc.vector.tensor_copy(Oacc, po[:, :D])
                        nc.vector.tensor_copy(lrun, lg_ap)
                    else:
                        nc.vector.scalar_tensor_tensor(
                            out=Oacc, in0=Oacc, scalar=cstep, in1=po[:, :D],
                            op0=ALU.mult, op1=ALU.add)
                        nc.vector.scalar_tensor_tensor(
                            out=lrun, in0=lrun, scalar=cstep, in1=lg_ap,
                            op0=ALU.mult, op1=ALU.add)

                rl = small.tile([P128, 1], f32, tag="rl")
                nc.vector.reciprocal(rl, lrun)
                yt = small.tile([P128, D], f32, tag="yt")
                nc.vector.tensor_scalar_mul(yt, Oacc, rl)
                nc.gpsimd.dma_start(out=y[j * P128:(j + 1) * P128, :], in_=yt)

    nc.compile()
    return nc


def _get_nc():
    if "nc" not in _CACHED:
        _CACHED["nc"] = _build()
    return _CACHED["nc"]


def _prep_in_maps(x, Wq, bq, Wk, bk, Wv, bv):
    x = np.asarray(x, dtype=np.float32)
    Wq = np.asarray(Wq, dtype=np.float32)
    Wk = np.asarray(Wk, dtype=np.float32)
    Wv = np.asarray(Wv, dtype=np.float32)
    bq_ = np.asarray(bq, dtype=np.float32).reshape(D, 1)
    bk_ = np.asarray(bk, dtype=np.float32).reshape(D, 1)
    bv_ = np.asarray(bv, dtype=np.float32).reshape(D, 1)

    tri = np.triu(np.ones((P128, P128), np.float32), k=1) * NEG
    masks = []
    for P in range(2):
        mp = np.zeros((P128, 512), np.float32)
        if P == 0:
            mp[:, 256:384] = tri
            mp[:, 384:512] = NEG
        else:
            mp[:, 384:512] = tri
        masks.append(mp)

    masksT = []
    for P in range(2):
        mt = np.zeros((P128, 256), np.float32)
        mt[:, 0:128] = masks[P][:, 256:384].T
        mt[:, 128:256] = masks[P][:, 384:512].T
        masksT.append(mt)

    in_maps = []
    for c in range(8):
        b, P = c // 2, c % 2
        xb = x[b]                                   # [T, C]
        rows = (np.arange(NSLOT) * 2 + P)[:, None] * P128 + np.arange(P128)[None, :]
        rows = rows.reshape(-1)
        in_maps.append({
            "xT": np.ascontiguousarray(xb.T),
            "xTq": np.ascontiguousarray(xb[rows].T),
            "wq": Wq, "wk": Wk, "wv": Wv,
            "bq": bq_, "bk": bk_, "bv": bv_,
            "maskp": masks[P], "maskpT": masksT[P],
        })
    return in_maps


def _unshard(res):
    out = np.empty((B, T, D), np.float32)
    for c in range(8):
        b, P = c // 2, c % 2
        yl = res.results[c]["y"]
        for j in range(NSLOT):
            k = 2 * j + P
            out[b, k * P128:(k + 1) * P128] = yl[j * P128:(j + 1) * P128]
    return out


def kernel(x, Wq, bq, Wk, bk, Wv, bv):
    from concourse.bass_utils import run_bass_kernel_spmd

    in_maps = _prep_in_maps(x, Wq, bq, Wk, bk, Wv, bv)
    res = run_bass_kernel_spmd(_get_nc(), in_maps, core_ids=list(range(8)))
    _CACHED["last_results"] = res
    return _unshard(res)


def run_profiled(np_inputs):
    from concourse.bass_utils import run_bass_kernel_spmd

    in_maps = _prep_in_maps(**np_inputs)
    res = run_bass_kernel_spmd(_get_nc(), in_maps, core_ids=list(range(8)),
                               trace=True)
    _CACHED["last_results"] = res
    return res


if __name__ == "__main__":
    rng = np.random.default_rng(0)
    x = rng.standard_normal((B, T, C), dtype=np.float32)
    s = 1.0 / np.sqrt(C)
    Wq = rng.standard_normal((C, D), dtype=np.float32) * s
    Wk = rng.standard_normal((C, D), dtype=np.float32) * s
    Wv = rng.standard_normal((C, D), dtype=np.float32) * s
    z = np.zeros(D, np.float32)
    print(kernel(x, Wq, z, Wk, z, Wv, z).shape)
